# revision 31
# baseline (speedup 1.0000x reference)
"""PSENet-style OHEM + dice loss on 8 Trainium2 NeuronCores — bf16 edition.

Data-parallel over the batch: core b processes image b entirely on-chip.
All inputs are cast to bf16 on the host (labels/masks are exactly 0/1 so
they are lossless; logits lose ~0.4% relative, far below the fp32 noise
floor of the 400k-element dice sums). This halves HBM traffic per image
from 24.5 MB to 12.25 MB — the kernel is HBM-bound at ~358 GB/s/core, so
the DMA stream (~34 us/image) sets the steady-state floor.

HW-measured per-pass costs at [128, 3200] bf16 (micro-benched on these
cores with bench.py — the CoreSim cost model is badly wrong for several):
  DVE tensor_tensor ~0.75us, tensor_scalar ~0.3us,
  scalar_tensor_tensor+accum ~1.75us (fused product+sum, 2x mode),
  ACT sigmoid ~1.1us, Square+accum ~1.5us, GPSIMD mul ~3.2us,
  tensor_scalar+accum ~2.7us (the reduce variant drops to 1x - avoided).

Schedule per kernel channel k (mask M = (x_text>0)*m, Mb = (2M-1)*BIG):
    xm = min(x, Mb)      DVE TT, in-place on the x tile
                         (sig(xm) = sig(x)*M up to sig(-BIG) ~ 2e-22)
    sb = sigmoid(xm)     ACT
    b  = sum sb^2        ACT Square + accum_out     (fp32 accumulators)
    a  = sum g*sb        DVE STT + accum_out
    c  = sum g*M         DVE STT + accum_out
Text channel: same with mask m (mb = (2m-1)*BIG); M = is_gt(xm6, 0) in a
single TS pass since xm6 > 0 <=> (x6>0 and m). pos_num = c_t feeds the
host-verified OHEM fast path. The last channel is processed as two
half-tiles so the post-final-DMA compute tail is halved. Channel heads
(DMA + xm + sigmoid) are emitted one channel ahead of tails (b/a/c sums)
so DVE's in-order queue never stalls on ACT.

Totals per image: DVE ~31us, ACT ~19us, DMA ~34us -> DMA-bound. accum
columns are cross-partition reduced by one ones-matmul per accumulator at
the end; the host combines 8 x 16x2 floats into the three scalars.
HW-measured: steady-state 32.6-35.1 us/image across runs (vs 72.4 us
for the fp32 predecessor), rel err 2.3e-06.

OHEM: for these inputs 3*pos_num >= total_neg for every image, so the
selected mask is exactly the training mask. The host VERIFIES
(RATIO+1)*pos_num >= N (pos_num = c_t, exact: 0/1 values, fp32 accum)
and falls back to a full host reference if it ever fails.

build_nc(reps, serial=True) emits a timing variant whose reps are
data-gated on the previous rep's accumulators (pipeline drained), used by
test.py to measure single-image latency with a large-signal difference.
"""

import os
import sys

import numpy as np

for _p in ("/opt/trn_rl_repo", "/root/.axon_site/_ro/trn_rl_repo"):
    if os.path.isdir(_p) and _p not in sys.path:
        sys.path.append(_p)

import concourse.bacc as bacc
import concourse.tile as tile
from concourse import mybir
from concourse.bass_utils import run_bass_kernel_spmd

B, C, H, W = 8, 7, 640, 640
NK = C - 1            # kernel channels
N = H * W             # pixels per image
P = 128               # SBUF partitions
F = N // P            # free dim per tile (3200)
BIG = 50.0
NCORES = 8
LAMBDA = 0.7
RATIO = 3

_dt = mybir.dt.bfloat16
_f8 = mybir.dt.float8e4
_f32 = mybir.dt.float32
_AF = mybir.ActivationFunctionType
_ALU = mybir.AluOpType

_NP_BF16 = None


def _np_bf16():
    global _NP_BF16
    if _NP_BF16 is None:
        _NP_BF16 = mybir.dt.np(mybir.dt.bfloat16)
    return _NP_BF16


def _img_ap(dram_ap):
    """[H, W] dram slab -> [128, 3200] partition-major access pattern."""
    return dram_ap.rearrange("(p q) w -> p (q w)", p=P)


def build_nc(debug=False, reps=1, serial=False):
    nc = bacc.Bacc("TRN2", target_bir_lowering=False, debug=debug)
    x_d = nc.dram_tensor("x", [C, H, W], _dt, kind="ExternalInput")
    # labels/mask are exactly 0/1: fp8 e4m3 is lossless and cuts their HBM
    # traffic in half again; ACT (dtype-independent, ~1.1us/pass, has
    # slack) upcasts them to bf16 for the DVE STT consumers
    g_d = nc.dram_tensor("g", [C, H, W], _f8, kind="ExternalInput")
    m_d = nc.dram_tensor("m", [H, W], _f8, kind="ExternalInput")
    res_d = nc.dram_tensor("res", [16, 2], _f32, kind="ExternalOutput")

    with (
        tile.TileContext(nc) as tc,
        tc.tile_pool(name="const", bufs=1) as cpool,
        tc.tile_pool(name="mask", bufs=2) as mkpool,
        tc.tile_pool(name="xin", bufs=5) as xpool,
        tc.tile_pool(name="gin", bufs=4) as gpool,
        tc.tile_pool(name="g8", bufs=3) as g8pool,
        tc.tile_pool(name="sbp", bufs=4) as sbpool,
        tc.tile_pool(name="ps", bufs=1, space="PSUM") as ppool,
    ):
        # fp32 per-partition accumulators (fused accum_out targets).
        # acc_dve (DVE STT): 0=a_t, 1..5=a_k(k=0..4), 6,7=a_5 halves,
        #                    8=c_t(=pos_num), 9..13=c_k(k=0..4), 14=c_5
        # acc_act (ACT Square): 0=b_t, 1..5=b_k(k=0..4), 6,7=b_5 halves
        acc_dve = cpool.tile([P, 16], _f32)
        acc_act = cpool.tile([P, 16], _f32)
        ones_f = cpool.tile([P, 1], _f32)
        nc.gpsimd.memset(ones_f[:], 1.0)
        nc.vector.memset(acc_dve[:], 0.0)
        nc.scalar.memzero(acc_act[:])
        # shared junk outputs for accum passes — one per engine, so the
        # WAW chains stay engine-internal (in-order: zero cost)
        ajunk = cpool.tile([P, F], _dt)
        djunk = cpool.tile([P, F], _dt)
        # serial-mode rep-serialization tokens (see below)
        tokD = cpool.tile([P, 1], _f32)
        tokA = cpool.tile([P, 1], _f32)
        tokX = cpool.tile([P, 1], _f32)

        def image_body(rep):
            Fh = F // 2
            heads = {}

            def head(k):
                xt = xpool.tile([P, F], _dt, tag="xin", name=f"xk{k}_r{rep}")
                gt = gpool.tile([P, F], _dt, tag="gin", name=f"gk{k}_r{rep}")
                if k == 6:
                    # text channel first: its logits gate everything
                    m_t = mkpool.tile([P, F], _dt, tag="m_t",
                                      name=f"m_t_r{rep}")
                    m8_t = mkpool.tile([P, F], _f8, tag="m8_t",
                                       name=f"m8_t_r{rep}")
                    if serial and rep > 0:
                        # gate: junk-write into the DMA target, reading the
                        # previous rep's tokens; the DMA queue is FIFO so
                        # this serializes the whole rep (values unused)
                        nc.vector.tensor_tensor(
                            tokX[:], tokD[:], tokA[:], _ALU.add)
                        nc.vector.tensor_tensor(
                            m8_t[:, 0:1], tokX[:], tokX[:], _ALU.mult)
                    nc.sync.dma_start(m8_t[:], _img_ap(m_d.ap()))
                    nc.sync.dma_start(xt[:], _img_ap(x_d.ap()[k]))
                    g8 = g8pool.tile([P, F], _f8, tag="g8",
                                     name=f"g8k{k}_r{rep}")
                    nc.sync.dma_start(g8[:], _img_ap(g_d.ap()[k]))
                    nc.scalar.copy(m_t[:], m8_t[:])
                    nc.scalar.copy(gt[:], g8[:])
                    mb_t = mkpool.tile([P, F], _dt, tag="mb_t",
                                       name=f"mb_t_r{rep}", bufs=1)
                    nc.vector.tensor_scalar(
                        mb_t[:], m_t[:], 2.0 * BIG, -BIG, _ALU.mult,
                        _ALU.add
                    )
                    nc.vector.tensor_tensor(xt[:], xt[:], mb_t[:], _ALU.min)
                    sbt = sbpool.tile([P, F], _dt, tag="sbp",
                                      name=f"sb{k}_r{rep}")
                    nc.scalar.activation(sbt[:], xt[:], _AF.Sigmoid)
                    # xm6 > 0  <=>  (x6 > 0) and m: M from one is_gt pass
                    M_t = mkpool.tile([P, F], _dt, tag="M_t",
                                      name=f"M_t_r{rep}")
                    Mb_t = mkpool.tile([P, F], _dt, tag="Mb_t",
                                       name=f"Mb_t_r{rep}")
                    nc.vector.tensor_scalar(M_t[:], xt[:], 0.0, 0.0,
                                            _ALU.is_gt, _ALU.add)
                    nc.vector.tensor_scalar(
                        Mb_t[:], M_t[:], 2.0 * BIG, -BIG, _ALU.mult,
                        _ALU.add
                    )
                    heads["masks"] = (m_t, M_t, Mb_t)
                    heads[k] = (xt, gt, sbt)
                elif k < 5:
                    m_t, M_t, Mb_t = heads["masks"]
                    nc.sync.dma_start(xt[:], _img_ap(x_d.ap()[k]))
                    g8 = g8pool.tile([P, F], _f8, tag="g8",
                                     name=f"g8k{k}_r{rep}")
                    nc.sync.dma_start(g8[:], _img_ap(g_d.ap()[k]))
                    nc.vector.tensor_tensor(xt[:], xt[:], Mb_t[:], _ALU.min)
                    sbt = sbpool.tile([P, F], _dt, tag="sbp",
                                      name=f"sb{k}_r{rep}")
                    nc.scalar.activation(sbt[:], xt[:], _AF.Sigmoid)
                    nc.scalar.copy(gt[:], g8[:])
                    heads[k] = (xt, gt, sbt)
                else:
                    # last channel in two half-tiles: short post-final-DMA
                    # tail (single-shot latency)
                    m_t, M_t, Mb_t = heads["masks"]
                    sbts = []
                    g8 = g8pool.tile([P, F], _f8, tag="g8",
                                     name=f"g8k{k}_r{rep}")
                    for h in range(2):
                        xs = xt[:, h * Fh:(h + 1) * Fh]
                        src = _img_ap(x_d.ap()[k])
                        nc.sync.dma_start(xs, src[:, h * Fh:(h + 1) * Fh])
                        srcg = _img_ap(g_d.ap()[k])
                        nc.sync.dma_start(g8[:, h * Fh:(h + 1) * Fh],
                                          srcg[:, h * Fh:(h + 1) * Fh])
                        nc.vector.tensor_tensor(
                            xs, xs, Mb_t[:, h * Fh:(h + 1) * Fh], _ALU.min
                        )
                        sbt = sbpool.tile([P, F], _dt, tag="sbp",
                                          name=f"sbL{h}_r{rep}")
                        nc.scalar.activation(sbt[:, :Fh], xs, _AF.Sigmoid)
                        nc.scalar.copy(gt[:, h * Fh:(h + 1) * Fh],
                                       g8[:, h * Fh:(h + 1) * Fh])
                        sbts.append(sbt)
                    heads[k] = (xt, gt, sbts)

            def tail(k):
                m_t, M_t, Mb_t = heads["masks"]
                if k == 6:
                    xt, gt, sbt = heads[k]
                    nc.scalar.activation(
                        ajunk[:], sbt[:], _AF.Square,
                        accum_out=acc_act[:, 0:1],
                    )
                    nc.vector.scalar_tensor_tensor(
                        djunk[:], gt[:], 1.0, sbt[:], _ALU.mult, _ALU.mult,
                        accum_out=acc_dve[:, 0:1],
                    )
                    nc.vector.scalar_tensor_tensor(
                        djunk[:], gt[:], 1.0, m_t[:], _ALU.mult, _ALU.mult,
                        accum_out=acc_dve[:, 8:9],
                    )
                elif k < 5:
                    xt, gt, sbt = heads[k]
                    nc.scalar.activation(
                        ajunk[:], sbt[:], _AF.Square,
                        accum_out=acc_act[:, 1 + k:2 + k],
                    )
                    nc.vector.scalar_tensor_tensor(
                        djunk[:], gt[:], 1.0, sbt[:], _ALU.mult, _ALU.mult,
                        accum_out=acc_dve[:, 1 + k:2 + k],
                    )
                    nc.vector.scalar_tensor_tensor(
                        djunk[:], gt[:], 1.0, M_t[:], _ALU.mult, _ALU.mult,
                        accum_out=acc_dve[:, 9 + k:10 + k],
                    )
                else:
                    xt, gt, sbts = heads[k]
                    for h in range(2):
                        sbt = sbts[h]
                        gs = gt[:, h * Fh:(h + 1) * Fh]
                        nc.scalar.activation(
                            ajunk[:, :Fh], sbt[:, :Fh], _AF.Square,
                            accum_out=acc_act[:, 6 + h:7 + h],
                        )
                        nc.vector.scalar_tensor_tensor(
                            djunk[:, :Fh], gs, 1.0, sbt[:, :Fh],
                            _ALU.mult, _ALU.mult,
                            accum_out=acc_dve[:, 6 + h:7 + h],
                        )
                    # c_5 on the full tile (both g halves have landed)
                    nc.vector.scalar_tensor_tensor(
                        djunk[:], gt[:], 1.0, M_t[:], _ALU.mult, _ALU.mult,
                        accum_out=acc_dve[:, 14:15],
                    )

            # software pipelining, skew 1 (ACT sigmoids are ~1.1us, so one
            # channel of lookahead keeps DVE's in-order queue fed)
            order = [6, 0, 1, 2, 3, 4, 5]
            pend = []
            for k in order:
                head(k)
                if pend:
                    tail(pend.pop(0))
                pend.append(k)
            for k in pend:
                tail(k)

        for rep in range(reps):
            image_body(rep)
            if serial and rep < reps - 1:
                # tokens covering each engine's rep work: DVE accums ->
                # tokD; ACT squares -> tokA (both engines in-order)
                nc.vector.tensor_scalar(tokD[:], acc_dve[:, 0:1], 1.0, 0.0,
                                        _ALU.mult, _ALU.add)
                nc.scalar.activation(tokA[:], acc_act[:, 0:1], _AF.Copy)

        # cross-partition reduction of the accum columns with one
        # ones-vector matmul per accumulator: res row i <- sum_p acc[p, i]
        pr = ppool.tile([16, 2], _f32, tag="pr")
        nc.tensor.matmul(pr[:, 0:1], lhsT=acc_dve[:], rhs=ones_f[:],
                         start=True, stop=True)
        nc.tensor.matmul(pr[:, 1:2], lhsT=acc_act[:], rhs=ones_f[:],
                         start=True, stop=True)
        res_sb = cpool.tile([16, 2], _f32)
        nc.scalar.copy(res_sb[:], pr[:])
        nc.sync.dma_start(res_d.ap(), res_sb[:])

    nc.compile()
    return nc


_CACHE = {}


def _get_nc():
    if "nc" not in _CACHE:
        _CACHE["nc"] = build_nc(debug=False)
    return _CACHE["nc"]


def _combine(res_list):
    """Per-image [16,2] device sums -> (loss_text, loss_kernels, loss).

    res col 0 = acc_dve (a and c sums), col 1 = acc_act (b sums); see
    build_nc for the column layout.
    Returns None if the OHEM fast-path precondition fails for any image.
    """
    lt_b = np.zeros(B, np.float64)
    lk_b = np.zeros(B, np.float64)
    for b in range(B):
        v = np.asarray(res_list[b], np.float64)
        a_t, b_t, c_t = v[0, 0], v[0, 1], v[8, 0]
        pos_num = c_t                    # sum(gt_text * m), exact integer
        # sel == m iff pos_num == 0 (fallback) or neg_num == total_neg,
        # i.e. RATIO*pos_num >= total_neg = N - sum_g. Since sum_g >=
        # sum_g*m = pos_num, (RATIO+1)*pos_num >= N is sufficient.
        if not (pos_num == 0 or (RATIO + 1) * pos_num >= N):
            return None
        lt_b[b] = 1.0 - 2.0 * a_t / (b_t + 0.001 + c_t + 0.001)
        lk = 0.0
        for k in range(NK):
            if k < 5:
                a_k, b_k, c_k = v[1 + k, 0], v[1 + k, 1], v[9 + k, 0]
            else:
                a_k = v[6, 0] + v[7, 0]
                b_k = v[6, 1] + v[7, 1]
                c_k = v[14, 0]
            lk += 1.0 - 2.0 * a_k / (b_k + 0.001 + c_k + 0.001)
        lk_b[b] = lk / NK
    lt = np.float32(lt_b.mean())
    lk = np.float32(lk_b.mean())
    loss = np.float32(LAMBDA) * lt + np.float32(1.0 - LAMBDA) * lk
    return (lt, lk, np.float32(loss))


def _numpy_reference(outputs, labels, training_masks):
    """Full-fidelity host fallback (mirrors the original loss exactly)."""
    def sigmoid(z):
        return 1.0 / (1.0 + np.exp(-z, dtype=np.float64))

    texts = outputs[:, -1].reshape(B, N).astype(np.float64)
    kernels = outputs[:, :-1].reshape(B, NK, N).astype(np.float64)
    gt_texts = labels[:, -1].reshape(B, N).astype(np.float64)
    gt_kernels = labels[:, :-1].reshape(B, NK, N).astype(np.float64)
    tm = training_masks.reshape(B, N).astype(np.float64)

    pos = gt_texts > 0.5
    pos_num = np.sum(pos & (tm > 0.5), axis=1)
    neg = ~pos
    total_neg = np.sum(neg, axis=1)
    neg_num = np.minimum(pos_num * RATIO, total_neg)
    neg_scores = np.where(neg, texts, -np.inf)
    sorted_desc = -np.sort(-neg_scores, axis=1)
    idx = np.clip(neg_num - 1, 0, N - 1)
    thr = np.take_along_axis(sorted_desc, idx[:, None], axis=1)
    sel = (((texts >= thr) | pos) & (tm > 0.5)).astype(np.float64)
    fallback = (pos_num == 0) | (neg_num == 0)
    sel = np.where(fallback[:, None], tm, sel)

    def dice(inp, target, mask):
        p = sigmoid(inp) * mask
        t = target * mask
        a = np.sum(p * t, axis=-1)
        bb = np.sum(p * p, axis=-1) + 0.001
        cc = np.sum(t * t, axis=-1) + 0.001
        return 1.0 - 2.0 * a / (bb + cc)

    loss_text = dice(texts, gt_texts, sel).mean()
    sel_k = ((sigmoid(texts) > 0.5) & (tm > 0.5)).astype(np.float64)
    loss_kernels = dice(kernels, gt_kernels, sel_k[:, None, :]).mean(axis=1).mean()
    loss = LAMBDA * loss_text + (1.0 - LAMBDA) * loss_kernels
    return (np.float32(loss_text), np.float32(loss_kernels), np.float32(loss))


def kernel(outputs, labels, training_masks):
    outputs = np.asarray(outputs, dtype=np.float32)
    labels = np.asarray(labels, dtype=np.float32)
    training_masks = np.asarray(training_masks, dtype=np.float32)
    assert outputs.shape == (B, C, H, W)

    bf16 = _np_bf16()
    fp8 = mybir.dt.np(_f8)
    nc = _get_nc()
    in_maps = [
        {
            "x": np.ascontiguousarray(outputs[b]).astype(bf16),
            "g": np.ascontiguousarray(labels[b]).astype(fp8),
            "m": np.ascontiguousarray(training_masks[b]).astype(fp8),
        }
        for b in range(B)
    ]
    r = None
    for attempt in range(3):
        try:
            r = run_bass_kernel_spmd(
                nc, in_maps, list(range(NCORES)),
                trace=_CACHE.get("trace", False),
            )
            break
        except Exception:
            if attempt == 2:
                raise
            _CACHE.pop("nc", None)
            nc = _get_nc()
    _CACHE["last_result"] = r
    res_list = [r.results[b]["res"] for b in range(B)]
    out = _combine(res_list)
    if out is None:
        # OHEM threshold is not the minimum negative score -> exact host path
        out = _numpy_reference(outputs, labels, training_masks)
    return out


# revision 34
# speedup vs baseline: 1.0353x; 1.0353x over previous
"""PSENet-style OHEM + dice loss on 8 Trainium2 NeuronCores — bf16 edition.

Data-parallel over the batch: core b processes image b entirely on-chip.
All inputs are cast to bf16 on the host (labels/masks are exactly 0/1 so
they are lossless; logits lose ~0.4% relative, far below the fp32 noise
floor of the 400k-element dice sums). This halves HBM traffic per image
from 24.5 MB to 12.25 MB — the kernel is HBM-bound at ~358 GB/s/core, so
the DMA stream (~34 us/image) sets the steady-state floor.

HW-measured per-pass costs at [128, 3200] bf16 (micro-benched on these
cores with bench.py — the CoreSim cost model is badly wrong for several):
  DVE tensor_tensor ~0.75us, tensor_scalar ~0.3us,
  scalar_tensor_tensor+accum ~1.75us (fused product+sum, 2x mode),
  ACT sigmoid ~1.1us, Square+accum ~1.5us, GPSIMD mul ~3.2us,
  tensor_scalar+accum ~2.7us (the reduce variant drops to 1x - avoided).

Schedule per kernel channel k (mask M = (x_text>0)*m, Mb = (2M-1)*BIG):
    xm = min(x, Mb)      DVE TT, in-place on the x tile
                         (sig(xm) = sig(x)*M up to sig(-BIG) ~ 2e-22)
    sb = sigmoid(xm)     ACT
    b  = sum sb^2        ACT Square + accum_out     (fp32 accumulators)
    a  = sum g*sb        DVE STT + accum_out
    c  = sum g*M         DVE STT + accum_out
Text channel: same with mask m (mb = (2m-1)*BIG); M = is_gt(xm6, 0) in a
single TS pass since xm6 > 0 <=> (x6>0 and m). pos_num = c_t feeds the
host-verified OHEM fast path. The last channel is processed as two
half-tiles so the post-final-DMA compute tail is halved. Channel heads
(DMA + xm + sigmoid) are emitted one channel ahead of tails (b/a/c sums)
so DVE's in-order queue never stalls on ACT.

Totals per image: DVE ~31us, ACT ~19us, DMA ~34us -> DMA-bound. accum
columns are cross-partition reduced by one ones-matmul per accumulator at
the end; the host combines 8 x 16x2 floats into the three scalars.
HW-measured: steady-state 32.6-35.1 us/image across runs (vs 72.4 us
for the fp32 predecessor), rel err 2.3e-06.

OHEM: for these inputs 3*pos_num >= total_neg for every image, so the
selected mask is exactly the training mask. The host VERIFIES
(RATIO+1)*pos_num >= N (pos_num = c_t, exact: 0/1 values, fp32 accum)
and falls back to a full host reference if it ever fails.

build_nc(reps, serial=True) emits a timing variant whose reps are
data-gated on the previous rep's accumulators (pipeline drained), used by
test.py to measure single-image latency with a large-signal difference.
"""

import os
import sys

import numpy as np

for _p in ("/opt/trn_rl_repo", "/root/.axon_site/_ro/trn_rl_repo"):
    if os.path.isdir(_p) and _p not in sys.path:
        sys.path.append(_p)

import concourse.bacc as bacc
import concourse.tile as tile
from concourse import mybir
from concourse.bass_utils import run_bass_kernel_spmd

B, C, H, W = 8, 7, 640, 640
NK = C - 1            # kernel channels
N = H * W             # pixels per image
P = 128               # SBUF partitions
F = N // P            # free dim per tile (3200)
BIG = 50.0
NCORES = 8
LAMBDA = 0.7
RATIO = 3

_dt = mybir.dt.bfloat16
_f32 = mybir.dt.float32
_AF = mybir.ActivationFunctionType
_ALU = mybir.AluOpType

_NP_BF16 = None


def _np_bf16():
    global _NP_BF16
    if _NP_BF16 is None:
        _NP_BF16 = mybir.dt.np(mybir.dt.bfloat16)
    return _NP_BF16


def _img_ap(dram_ap):
    """[H, W] dram slab -> [128, 3200] partition-major access pattern."""
    return dram_ap.rearrange("(p q) w -> p (q w)", p=P)


def build_nc(debug=False, reps=1, serial=False):
    nc = bacc.Bacc("TRN2", target_bir_lowering=False, debug=debug)
    x_d = nc.dram_tensor("x", [C, H, W], _dt, kind="ExternalInput")
    g_d = nc.dram_tensor("g", [C, H, W], _dt, kind="ExternalInput")
    m_d = nc.dram_tensor("m", [H, W], _dt, kind="ExternalInput")
    res_d = nc.dram_tensor("res", [16, 2], _f32, kind="ExternalOutput")

    with (
        tile.TileContext(nc) as tc,
        tc.tile_pool(name="const", bufs=1) as cpool,
        tc.tile_pool(name="mask", bufs=2) as mkpool,
        tc.tile_pool(name="xin", bufs=5) as xpool,
        tc.tile_pool(name="gin", bufs=5) as gpool,
        tc.tile_pool(name="sbp", bufs=4) as sbpool,
        tc.tile_pool(name="ps", bufs=1, space="PSUM") as ppool,
    ):
        # fp32 per-partition accumulators (fused accum_out targets).
        # acc_dve (DVE STT): 0=a_t, 1..5=a_k(k=0..4), 6,7=a_5 halves,
        #                    8=c_t(=pos_num), 9..13=c_k(k=0..4), 14=c_5
        # acc_act (ACT Square): 0=b_t, 1..5=b_k(k=0..4), 6,7=b_5 halves
        acc_dve = cpool.tile([P, 16], _f32)
        acc_act = cpool.tile([P, 16], _f32)
        ones_f = cpool.tile([P, 1], _f32)
        nc.gpsimd.memset(ones_f[:], 1.0)
        nc.vector.memset(acc_dve[:], 0.0)
        nc.scalar.memzero(acc_act[:])
        # shared junk outputs for accum passes — one per engine, so the
        # WAW chains stay engine-internal (in-order: zero cost)
        ajunk = cpool.tile([P, F], _dt)
        djunk = cpool.tile([P, F], _dt)
        # serial-mode rep-serialization tokens (see below)
        tokD = cpool.tile([P, 1], _f32)
        tokA = cpool.tile([P, 1], _f32)
        tokX = cpool.tile([P, 1], _f32)

        def image_body(rep):
            Fh = F // 2
            heads = {}

            def head(k):
                xt = xpool.tile([P, F], _dt, tag="xin", name=f"xk{k}_r{rep}")
                gt = gpool.tile([P, F], _dt, tag="gin", name=f"gk{k}_r{rep}")
                if k == 6:
                    # text channel first: its logits gate everything
                    m_t = mkpool.tile([P, F], _dt, tag="m_t",
                                      name=f"m_t_r{rep}")
                    if serial and rep > 0:
                        # gate: junk-write into the DMA target, reading the
                        # previous rep's tokens; the DMA queue is FIFO so
                        # this serializes the whole rep (values unused)
                        nc.vector.tensor_tensor(
                            tokX[:], tokD[:], tokA[:], _ALU.add)
                        nc.vector.tensor_tensor(
                            m_t[:, 0:1], tokX[:], tokX[:], _ALU.mult)
                    nc.sync.dma_start(m_t[:], _img_ap(m_d.ap()))
                    nc.sync.dma_start(xt[:], _img_ap(x_d.ap()[k]))
                    nc.sync.dma_start(gt[:], _img_ap(g_d.ap()[k]))
                    mb_t = mkpool.tile([P, F], _dt, tag="mb_t",
                                       name=f"mb_t_r{rep}", bufs=1)
                    nc.vector.tensor_scalar(
                        mb_t[:], m_t[:], 2.0 * BIG, -BIG, _ALU.mult,
                        _ALU.add
                    )
                    nc.vector.tensor_tensor(xt[:], xt[:], mb_t[:], _ALU.min)
                    sbt = sbpool.tile([P, F], _dt, tag="sbp",
                                      name=f"sb{k}_r{rep}")
                    nc.scalar.activation(sbt[:], xt[:], _AF.Sigmoid)
                    # xm6 > 0  <=>  (x6 > 0) and m: M from one is_gt pass
                    M_t = mkpool.tile([P, F], _dt, tag="M_t",
                                      name=f"M_t_r{rep}")
                    Mb_t = mkpool.tile([P, F], _dt, tag="Mb_t",
                                       name=f"Mb_t_r{rep}")
                    nc.vector.tensor_scalar(M_t[:], xt[:], 0.0, 0.0,
                                            _ALU.is_gt, _ALU.add)
                    nc.vector.tensor_scalar(
                        Mb_t[:], M_t[:], 2.0 * BIG, -BIG, _ALU.mult,
                        _ALU.add
                    )
                    heads["masks"] = (m_t, M_t, Mb_t)
                    heads[k] = (xt, gt, sbt)
                elif k < 5:
                    m_t, M_t, Mb_t = heads["masks"]
                    nc.sync.dma_start(xt[:], _img_ap(x_d.ap()[k]))
                    nc.sync.dma_start(gt[:], _img_ap(g_d.ap()[k]))
                    nc.vector.tensor_tensor(xt[:], xt[:], Mb_t[:], _ALU.min)
                    sbt = sbpool.tile([P, F], _dt, tag="sbp",
                                      name=f"sb{k}_r{rep}")
                    nc.scalar.activation(sbt[:], xt[:], _AF.Sigmoid)
                    heads[k] = (xt, gt, sbt)
                else:
                    # last channel in two half-tiles: short post-final-DMA
                    # tail (single-shot latency)
                    m_t, M_t, Mb_t = heads["masks"]
                    sbts = []
                    for h in range(2):
                        xs = xt[:, h * Fh:(h + 1) * Fh]
                        src = _img_ap(x_d.ap()[k])
                        nc.sync.dma_start(xs, src[:, h * Fh:(h + 1) * Fh])
                        srcg = _img_ap(g_d.ap()[k])
                        nc.sync.dma_start(gt[:, h * Fh:(h + 1) * Fh],
                                          srcg[:, h * Fh:(h + 1) * Fh])
                        nc.vector.tensor_tensor(
                            xs, xs, Mb_t[:, h * Fh:(h + 1) * Fh], _ALU.min
                        )
                        sbt = sbpool.tile([P, F], _dt, tag="sbp",
                                          name=f"sbL{h}_r{rep}")
                        nc.scalar.activation(sbt[:, :Fh], xs, _AF.Sigmoid)
                        sbts.append(sbt)
                    heads[k] = (xt, gt, sbts)

            def tail(k):
                m_t, M_t, Mb_t = heads["masks"]
                if k == 6:
                    xt, gt, sbt = heads[k]
                    nc.scalar.activation(
                        ajunk[:], sbt[:], _AF.Square,
                        accum_out=acc_act[:, 0:1],
                    )
                    nc.vector.scalar_tensor_tensor(
                        djunk[:], gt[:], 1.0, sbt[:], _ALU.mult, _ALU.mult,
                        accum_out=acc_dve[:, 0:1],
                    )
                    nc.vector.scalar_tensor_tensor(
                        djunk[:], gt[:], 1.0, m_t[:], _ALU.mult, _ALU.mult,
                        accum_out=acc_dve[:, 8:9],
                    )
                elif k < 5:
                    xt, gt, sbt = heads[k]
                    nc.scalar.activation(
                        ajunk[:], sbt[:], _AF.Square,
                        accum_out=acc_act[:, 1 + k:2 + k],
                    )
                    nc.vector.scalar_tensor_tensor(
                        djunk[:], gt[:], 1.0, sbt[:], _ALU.mult, _ALU.mult,
                        accum_out=acc_dve[:, 1 + k:2 + k],
                    )
                    nc.vector.scalar_tensor_tensor(
                        djunk[:], gt[:], 1.0, M_t[:], _ALU.mult, _ALU.mult,
                        accum_out=acc_dve[:, 9 + k:10 + k],
                    )
                else:
                    xt, gt, sbts = heads[k]
                    for h in range(2):
                        sbt = sbts[h]
                        gs = gt[:, h * Fh:(h + 1) * Fh]
                        nc.scalar.activation(
                            ajunk[:, :Fh], sbt[:, :Fh], _AF.Square,
                            accum_out=acc_act[:, 6 + h:7 + h],
                        )
                        nc.vector.scalar_tensor_tensor(
                            djunk[:, :Fh], gs, 1.0, sbt[:, :Fh],
                            _ALU.mult, _ALU.mult,
                            accum_out=acc_dve[:, 6 + h:7 + h],
                        )
                    # c_5 on the full tile (both g halves have landed)
                    nc.vector.scalar_tensor_tensor(
                        djunk[:], gt[:], 1.0, M_t[:], _ALU.mult, _ALU.mult,
                        accum_out=acc_dve[:, 14:15],
                    )

            # software pipelining, skew 1 (ACT sigmoids are ~1.1us, so one
            # channel of lookahead keeps DVE's in-order queue fed)
            order = [6, 0, 1, 2, 3, 4, 5]
            pend = []
            for k in order:
                head(k)
                if pend:
                    tail(pend.pop(0))
                pend.append(k)
            for k in pend:
                tail(k)

        for rep in range(reps):
            image_body(rep)
            if serial and rep < reps - 1:
                # tokens covering each engine's rep work: DVE accums ->
                # tokD; ACT squares -> tokA (both engines in-order)
                nc.vector.tensor_scalar(tokD[:], acc_dve[:, 0:1], 1.0, 0.0,
                                        _ALU.mult, _ALU.add)
                nc.scalar.activation(tokA[:], acc_act[:, 0:1], _AF.Copy)

        # cross-partition reduction of the accum columns with one
        # ones-vector matmul per accumulator: res row i <- sum_p acc[p, i]
        pr = ppool.tile([16, 2], _f32, tag="pr")
        nc.tensor.matmul(pr[:, 0:1], lhsT=acc_dve[:], rhs=ones_f[:],
                         start=True, stop=True)
        nc.tensor.matmul(pr[:, 1:2], lhsT=acc_act[:], rhs=ones_f[:],
                         start=True, stop=True)
        res_sb = cpool.tile([16, 2], _f32)
        nc.scalar.copy(res_sb[:], pr[:])
        nc.sync.dma_start(res_d.ap(), res_sb[:])

    nc.compile()
    return nc


_CACHE = {}


def _get_nc():
    if "nc" not in _CACHE:
        _CACHE["nc"] = build_nc(debug=False)
    return _CACHE["nc"]


def _combine(res_list):
    """Per-image [16,2] device sums -> (loss_text, loss_kernels, loss).

    res col 0 = acc_dve (a and c sums), col 1 = acc_act (b sums); see
    build_nc for the column layout.
    Returns None if the OHEM fast-path precondition fails for any image.
    """
    lt_b = np.zeros(B, np.float64)
    lk_b = np.zeros(B, np.float64)
    for b in range(B):
        v = np.asarray(res_list[b], np.float64)
        a_t, b_t, c_t = v[0, 0], v[0, 1], v[8, 0]
        pos_num = c_t                    # sum(gt_text * m), exact integer
        # sel == m iff pos_num == 0 (fallback) or neg_num == total_neg,
        # i.e. RATIO*pos_num >= total_neg = N - sum_g. Since sum_g >=
        # sum_g*m = pos_num, (RATIO+1)*pos_num >= N is sufficient.
        if not (pos_num == 0 or (RATIO + 1) * pos_num >= N):
            return None
        lt_b[b] = 1.0 - 2.0 * a_t / (b_t + 0.001 + c_t + 0.001)
        lk = 0.0
        for k in range(NK):
            if k < 5:
                a_k, b_k, c_k = v[1 + k, 0], v[1 + k, 1], v[9 + k, 0]
            else:
                a_k = v[6, 0] + v[7, 0]
                b_k = v[6, 1] + v[7, 1]
                c_k = v[14, 0]
            lk += 1.0 - 2.0 * a_k / (b_k + 0.001 + c_k + 0.001)
        lk_b[b] = lk / NK
    lt = np.float32(lt_b.mean())
    lk = np.float32(lk_b.mean())
    loss = np.float32(LAMBDA) * lt + np.float32(1.0 - LAMBDA) * lk
    return (lt, lk, np.float32(loss))


def _numpy_reference(outputs, labels, training_masks):
    """Full-fidelity host fallback (mirrors the original loss exactly)."""
    def sigmoid(z):
        return 1.0 / (1.0 + np.exp(-z, dtype=np.float64))

    texts = outputs[:, -1].reshape(B, N).astype(np.float64)
    kernels = outputs[:, :-1].reshape(B, NK, N).astype(np.float64)
    gt_texts = labels[:, -1].reshape(B, N).astype(np.float64)
    gt_kernels = labels[:, :-1].reshape(B, NK, N).astype(np.float64)
    tm = training_masks.reshape(B, N).astype(np.float64)

    pos = gt_texts > 0.5
    pos_num = np.sum(pos & (tm > 0.5), axis=1)
    neg = ~pos
    total_neg = np.sum(neg, axis=1)
    neg_num = np.minimum(pos_num * RATIO, total_neg)
    neg_scores = np.where(neg, texts, -np.inf)
    sorted_desc = -np.sort(-neg_scores, axis=1)
    idx = np.clip(neg_num - 1, 0, N - 1)
    thr = np.take_along_axis(sorted_desc, idx[:, None], axis=1)
    sel = (((texts >= thr) | pos) & (tm > 0.5)).astype(np.float64)
    fallback = (pos_num == 0) | (neg_num == 0)
    sel = np.where(fallback[:, None], tm, sel)

    def dice(inp, target, mask):
        p = sigmoid(inp) * mask
        t = target * mask
        a = np.sum(p * t, axis=-1)
        bb = np.sum(p * p, axis=-1) + 0.001
        cc = np.sum(t * t, axis=-1) + 0.001
        return 1.0 - 2.0 * a / (bb + cc)

    loss_text = dice(texts, gt_texts, sel).mean()
    sel_k = ((sigmoid(texts) > 0.5) & (tm > 0.5)).astype(np.float64)
    loss_kernels = dice(kernels, gt_kernels, sel_k[:, None, :]).mean(axis=1).mean()
    loss = LAMBDA * loss_text + (1.0 - LAMBDA) * loss_kernels
    return (np.float32(loss_text), np.float32(loss_kernels), np.float32(loss))


def kernel(outputs, labels, training_masks):
    outputs = np.asarray(outputs, dtype=np.float32)
    labels = np.asarray(labels, dtype=np.float32)
    training_masks = np.asarray(training_masks, dtype=np.float32)
    assert outputs.shape == (B, C, H, W)

    bf16 = _np_bf16()
    nc = _get_nc()
    in_maps = [
        {
            "x": np.ascontiguousarray(outputs[b]).astype(bf16),
            "g": np.ascontiguousarray(labels[b]).astype(bf16),
            "m": np.ascontiguousarray(training_masks[b]).astype(bf16),
        }
        for b in range(B)
    ]
    r = None
    for attempt in range(3):
        try:
            r = run_bass_kernel_spmd(
                nc, in_maps, list(range(NCORES)),
                trace=_CACHE.get("trace", False),
            )
            break
        except Exception:
            if attempt == 2:
                raise
            _CACHE.pop("nc", None)
            nc = _get_nc()
    _CACHE["last_result"] = r
    res_list = [r.results[b]["res"] for b in range(B)]
    out = _combine(res_list)
    if out is None:
        # OHEM threshold is not the minimum negative score -> exact host path
        out = _numpy_reference(outputs, labels, training_masks)
    return out
